# revision 1
# baseline (speedup 1.0000x reference)
"""ConvDeepSet Trainium2 kernel.

Computes, for each batch b:
    d2[n,m]   = (c[n] - t[m])^2                          (PE matmul, K small)
    w[n,m]    = exp(s * d2[n,m])                          (ACT exp, s = -0.5/scale^2)
    out1[c,m] = sum_n ctx[n,c] * w[n,m]                   (PE matmul, accumulate over n)
    density   = out1 row for the ones-channel
    conv_c    = out1 rows for feature channels
    res[m,o]  = W0[o]*density[m] + b[o]
                + (sum_c WT[c,o]*conv_c[m]) / (density[m] + 1e-8)
Sharded data-parallel over B across 8 NeuronCores (2 batches per core).
"""

import sys

if "/opt/trn_rl_repo" not in sys.path:
    sys.path.insert(0, "/opt/trn_rl_repo")

import numpy as np
import ml_dtypes

import concourse.bass as bass
import concourse.bacc as bacc
import concourse.tile as tile
import concourse.mybir as mybir
from concourse.bass_utils import run_bass_kernel_spmd

B, N, M, CIN, COUT = 16, 512, 1024, 7, 64
C = CIN + 1
N_CORES = 8
BPC = B // N_CORES  # batches per core
NT = N // 128       # n-tiles per batch
F32 = mybir.dt.float32
F32R = mybir.dt.float32r
F16 = mybir.dt.float16
BF16 = mybir.dt.bfloat16

# (lhsT-part, rhs-part) index pairs for the bf16 3-way-split cross terms of -2*c*t
_SPLIT_PAIRS = [(0, 0), (0, 1), (1, 0), (0, 2), (2, 0), (1, 1), (1, 2), (2, 1)]
SPLIT_K = 6 + len(_SPLIT_PAIRS)  # 3 (c^2 rows) + 3 (t^2 rows) + cross terms


def _build(svals, diff_mode, mm1_mode, epi_bcast, reps=1, diff_pack=True):
    """Build the SPMD Bass program. svals: tuple of per-group exp scales."""
    G = len(svals)
    KD = SPLIT_K if diff_mode == "split" else 3
    d_dt = BF16 if diff_mode == "split" else F32
    w_dt = {"f16": F16, "f32r": F32R, "f32": F32}[mm1_mode]

    nc = bacc.Bacc("TRN2", target_bir_lowering=False, debug=False,
                   num_devices=N_CORES)

    ctx_io_dt = F32 if mm1_mode == "f32r" else w_dt
    # packed diff layout: n-tile k sits at partition base 32*(k%2), pair k//2
    if diff_pack:
        L_d = nc.dram_tensor("L", [BPC, 32 + KD, NT // 2, 128], d_dt,
                             kind="ExternalInput")
        R_d = nc.dram_tensor("R", [BPC, 32 + KD, M], d_dt,
                             kind="ExternalInput")
    else:
        L_d = nc.dram_tensor("L", [BPC, KD, NT, 128], d_dt,
                             kind="ExternalInput")
        R_d = nc.dram_tensor("R", [BPC, KD, M], d_dt, kind="ExternalInput")
    ctx_d = nc.dram_tensor("ctx", [BPC, 128, G * NT * C], ctx_io_dt,
                           kind="ExternalInput")
    ones_d = nc.dram_tensor("ones", [1, M], F32, kind="ExternalInput")
    rb_d = nc.dram_tensor("rb", [C + 1, COUT], F32, kind="ExternalInput")
    ra_d = nc.dram_tensor("ra", [C + 1, COUT + 1], F32, kind="ExternalInput")
    out_d = nc.dram_tensor("out", [BPC, M, COUT], F32, kind="ExternalOutput")

    def mm_cast_mm1(ap):
        return ap.bitcast(F32R) if mm1_mode == "f32r" else ap

    with tile.TileContext(nc) as tc:
        with (
            tc.tile_pool(name="const", bufs=1) as constp,
            tc.tile_pool(name="inp", bufs=2) as inp,
            tc.tile_pool(name="wp", bufs=3) as wp,
            tc.tile_pool(name="o1p", bufs=2) as o1p,
            tc.tile_pool(name="resp", bufs=2) as resp,
            tc.tile_pool(name="rcp", bufs=2) as rcp,
            tc.tile_pool(name="dps", bufs=2, space=bass.MemorySpace.PSUM) as dps,
            tc.tile_pool(name="o1ps", bufs=1, space=bass.MemorySpace.PSUM) as o1ps,
            tc.tile_pool(name="aps", bufs=1, space=bass.MemorySpace.PSUM) as aps,
            tc.tile_pool(name="bps", bufs=1, space=bass.MemorySpace.PSUM) as bps,
        ):
            rb_t = constp.tile([C + 1, COUT], F32, tag="rb")
            nc.sync.dma_start(rb_t[:], rb_d.ap())
            ra_t = constp.tile([C + 1, COUT + 1], F32, tag="ra")
            nc.sync.dma_start(ra_t[:], ra_d.ap())

            def emit_phase1(j):
                lshape = ([32 + KD, NT // 2, 128] if diff_pack
                          else [KD, NT, 128])
                L_t = inp.tile(lshape, d_dt, tag="L")
                nc.sync.dma_start(L_t[:], L_d.ap()[j])
                R_t = inp.tile([32 + KD, M] if diff_pack else [KD, M],
                               d_dt, tag="R")
                nc.sync.dma_start(R_t[:], R_d.ap()[j])
                ctx_t = inp.tile([128, G, NT, C], ctx_io_dt, tag="ctx")
                nc.sync.dma_start(
                    ctx_t[:],
                    ctx_d.ap()[j].rearrange("p (g k c) -> p g k c",
                                            g=G, k=NT),
                )

                o1_t = o1ps.tile([C, M], F32, tag="o1")
                for k in range(NT):
                    base = 32 * (k % 2) if diff_pack else 0
                    d_t = dps.tile([128, M], F32, tag="d")
                    lhsT = (L_t[base:base + KD, k // 2, :] if diff_pack
                            else L_t[:, k, :])
                    for h in range(2):
                        nc.tensor.matmul(
                            d_t[:, h * 512:(h + 1) * 512],
                            lhsT,
                            R_t[base:base + KD, h * 512:(h + 1) * 512],
                            start=True, stop=True,
                            tile_position=(base, 0) if diff_pack else None,
                        )
                    for g in range(G):
                        w_t = wp.tile([128, M], w_dt, tag="w")
                        nc.scalar.activation(
                            w_t[:], d_t[:],
                            mybir.ActivationFunctionType.Exp,
                            scale=float(svals[g]),
                        )
                        first = (k == 0 and g == 0)
                        last = (k == NT - 1 and g == G - 1)
                        for h in range(2):
                            nc.tensor.matmul(
                                o1_t[:, h * 512:(h + 1) * 512],
                                mm_cast_mm1(ctx_t[:, g, k, :]),
                                mm_cast_mm1(w_t[:, h * 512:(h + 1) * 512]),
                                start=first, stop=last,
                            )
                return j, o1_t

            def emit_epilogue(j, o1_t):
                # division by density + final linear, m blocked as
                # m = 8*p + kk (partition p, group kk)
                o1_sb = o1p.tile([C + 1, M], F32, tag="o1sb")
                nc.vector.tensor_copy(o1_sb[0:C, :], o1_t[:])
                nc.sync.dma_start(o1_sb[C:C + 1, :], ones_d.ap())
                o1_g = o1_sb[:].rearrange("p (m q) -> p q m", q=8)

                res_t = resp.tile([128, 8 * COUT], F32, tag="res")
                for wave in range(2):
                    a_t = aps.tile([128, 4 * (COUT + 1)], F32, tag="a")
                    b_t = bps.tile([128, 4 * COUT], F32, tag="b")
                    for g4 in range(4):
                        kk = wave * 4 + g4
                        lhsT9 = o1_g[:, kk, :]
                        nc.tensor.matmul(
                            b_t[:, g4 * COUT:(g4 + 1) * COUT],
                            lhsT9, rb_t[:], start=True, stop=True,
                        )
                        nc.tensor.matmul(
                            a_t[:, g4 * (COUT + 1):(g4 + 1) * (COUT + 1)],
                            lhsT9, ra_t[:], start=True, stop=True,
                        )
                    a_g = a_t[:].rearrange("p (g x) -> p g x", x=COUT + 1)
                    recip_t = rcp.tile([128, 4], F32, tag="recip")
                    nc.vector.reciprocal(recip_t[:], a_g[:, :, COUT])
                    res_g = (res_t[:, wave * 4 * COUT:(wave + 1) * 4 * COUT]
                             .rearrange("p (g x) -> p g x", x=COUT))
                    if epi_bcast:
                        rb_ap = recip_t[:].unsqueeze(2).broadcast_to([128, 4, COUT])
                        nc.vector.tensor_tensor(
                            res_g, b_t[:].rearrange("p (g x) -> p g x", x=COUT),
                            rb_ap, mybir.AluOpType.mult,
                        )
                    else:
                        for g4 in range(4):
                            nc.vector.tensor_scalar_mul(
                                res_t[:, g4 * COUT:(g4 + 1) * COUT],
                                b_t[:, g4 * COUT:(g4 + 1) * COUT],
                                recip_t[:, g4:g4 + 1],
                            )
                    nc.vector.tensor_add(res_g, res_g, a_g[:, :, 0:COUT])
                nc.sync.dma_start(
                    out_d.ap()[j].rearrange("(p q) o -> p (q o)", q=8),
                    res_t[:],
                )

            pending = None
            for rep_j in range(reps * BPC):
                st = emit_phase1(rep_j % BPC)
                if pending is not None:
                    emit_epilogue(*pending)
                pending = st
            emit_epilogue(*pending)

    nc.compile()
    return nc


_CACHE = {}


def _get_program(svals, diff_mode, mm1_mode, epi_bcast, reps=1,
                 diff_pack=True):
    key = (tuple(np.float32(svals).tolist()), diff_mode, mm1_mode, epi_bcast,
           reps, diff_pack)
    if key not in _CACHE:
        _CACHE[key] = _build(svals, diff_mode, mm1_mode, epi_bcast, reps,
                             diff_pack)
    return _CACHE[key]


def _split3(x64):
    """Split float64 array into 3 bf16 arrays summing to ~fp32 accuracy."""
    parts = []
    r = x64.copy()
    for _ in range(3):
        p = r.astype(np.float32).astype(ml_dtypes.bfloat16)
        parts.append(p)
        r = r - p.astype(np.float64)
    return parts


def _host_prep(context_in, context_out, target_in, sigma, W, b,
               diff_mode, mm1_mode, diff_pack=True):
    ci = np.ascontiguousarray(np.asarray(context_in, np.float32)[:, :, 0])
    ti = np.ascontiguousarray(np.asarray(target_in, np.float32)[:, :, 0])
    co = np.asarray(context_out, np.float32)
    sig = np.asarray(sigma, np.float32)
    W = np.asarray(W, np.float32)
    bb = np.asarray(b, np.float32)

    scales = np.exp(sig.astype(np.float64))
    svals = (-0.5 / scales ** 2).astype(np.float32)
    uniq, inv = np.unique(svals, return_inverse=True)
    G = len(uniq)

    c64 = ci.astype(np.float64)
    t64 = ti.astype(np.float64)
    if diff_mode == "split":
        dt_np = ml_dtypes.bfloat16
        c_p = _split3(c64)
        t_p = _split3(t64)
        c2_p = _split3(c64 ** 2)
        t2_p = _split3(t64 ** 2)
        onesN = np.ones((B, N), dt_np)
        onesM = np.ones((B, M), dt_np)
        Lrows = c2_p + [onesN] * 3
        Rrows = [onesM] * 3 + t2_p
        for (i, jj) in _SPLIT_PAIRS:
            Lrows.append(c_p[i])
            Rrows.append((-2.0 * t_p[jj].astype(np.float32)).astype(dt_np))
        Lflat = np.stack(Lrows, axis=1)      # (B, SPLIT_K, N)
        Rflat = np.stack(Rrows, axis=1)      # (B, SPLIT_K, M)
        KD = SPLIT_K
    else:
        dt_np = np.float32
        Lflat = np.stack([c64 ** 2, -2.0 * c64, np.ones_like(c64)],
                         axis=1).astype(np.float32)
        Rflat = np.stack([np.ones_like(t64), t64, t64 ** 2],
                         axis=1).astype(np.float32)
        KD = 3
    # pack for row-group-concurrent diff matmuls: n-tile k at partition
    # base 32*(k%2), pair index k//2
    Lt = Lflat.reshape(B, KD, NT, 128)
    if diff_pack:
        L = np.zeros((B, 32 + KD, NT // 2, 128), dt_np)
        R = np.zeros((B, 32 + KD, M), dt_np)
        for k in range(NT):
            base = 32 * (k % 2)
            L[:, base:base + KD, k // 2, :] = Lt[:, :, k, :]
        R[:, 0:KD, :] = Rflat
        R[:, 32:32 + KD, :] = Rflat
    else:
        L = np.ascontiguousarray(Lt)
        R = np.ascontiguousarray(Rflat)

    w_np = np.float16 if mm1_mode == "f16" else np.float32
    ctx = np.zeros((B, G, N, C), w_np)
    for ch in range(C):
        g = int(inv[ch])
        if ch == 0:
            ctx[:, g, :, C - 1] = 1.0
        else:
            ctx[:, g, :, ch - 1] = co[:, :, ch - 1].astype(w_np)
    # device layout: partition p holds (g, k, c) contiguous
    ctx = np.ascontiguousarray(
        ctx.reshape(B, G, NT, 128, C).transpose(0, 3, 1, 2, 4)
        .reshape(B, 128, G * NT * C))

    # rb rows 0..6: W[:, 1:8].T ; rows 7,8: zero
    rb = np.zeros((C + 1, COUT), np.float32)
    rb[0:CIN, :] = W[:, 1:C].T
    # ra row 7: [W[:,0], 1]; row 8: [b, 1e-8]; rows 0..6: zero
    ra = np.zeros((C + 1, COUT + 1), np.float32)
    ra[C - 1, 0:COUT] = W[:, 0]
    ra[C - 1, COUT] = 1.0
    ra[C, 0:COUT] = bb
    ra[C, COUT] = 1e-8

    onesrow = np.ones((1, M), np.float32)

    in_maps = []
    for core in range(N_CORES):
        sl = slice(core * BPC, (core + 1) * BPC)
        in_maps.append({
            "L": np.ascontiguousarray(L[sl]),
            "R": np.ascontiguousarray(R[sl]),
            "ctx": np.ascontiguousarray(ctx[sl]),
            "ones": onesrow,
            "rb": rb,
            "ra": ra,
        })
    return uniq, in_maps


DIFF_MODE = "split"
MM1_MODE = "f16"
EPI_BCAST = True
DIFF_PACK = False


def kernel(context_in, context_out, target_in, sigma, W, b,
           diff_mode=None, mm1_mode=None, epi_bcast=None, trace=False,
           diff_pack=None):
    diff_mode = diff_mode or DIFF_MODE
    mm1_mode = mm1_mode or MM1_MODE
    epi_bcast = EPI_BCAST if epi_bcast is None else epi_bcast
    diff_pack = DIFF_PACK if diff_pack is None else diff_pack

    uniq_svals, in_maps = _host_prep(
        context_in, context_out, target_in, sigma, W, b, diff_mode, mm1_mode,
        diff_pack)
    nc = _get_program(tuple(uniq_svals.tolist()), diff_mode, mm1_mode,
                      epi_bcast, 1, diff_pack)
    res = run_bass_kernel_spmd(nc, in_maps, core_ids=list(range(N_CORES)),
                               trace=trace)
    out = np.concatenate([res.results[i]["out"] for i in range(N_CORES)],
                         axis=0)
    if trace:
        kernel.last_exec_time_ns = res.exec_time_ns
        kernel.last_results = res
    return out



# revision 13
# speedup vs baseline: 3.4698x; 3.4698x over previous
"""ConvDeepSet Trainium2 kernel (low-rank fast path).

Mathematical identity: with a single length-scale ell (sigma is a constant
vector in this model), the RBF weight matrix
    w[n, m] = exp(-(c_n - t_m)^2 / (2 ell^2)),   c, t in [0, 1]
is an analytic 1-D kernel, hence numerically low rank.  We build a
tensor-product Chebyshev interpolant of w on [a,b]^2 (degree D), SVD-truncate
its D x D core to rank R, and fold the factors into per-point features
    w ~= Phi(c) @ Psi(t)^T,    Phi: (N, R), Psi: (M, R)
computed on the host (pure polynomial evaluation; the transcendental part is
an input-independent constant folded into the features).  The device then
computes, per batch:
    P_Tt[c', r]  = sum_n ctxw[n, c'] Phi[n, r]          (PE, contraction N)
    PWA[r, :]    = P_Tt^T @ WFULL                        (PE, f32r, tiny)
    b_t[m, o]    = sum_r Psi_aug[r, m] PWA_sb[r, o]      (PE, chunked lhsT)
    a_t[m, o']   = sum_r Psi_aug[r, m] PWA_sb[r, 64+o']
    res[m, o]    = b_t[m, o] / a_t[m, 64] + a_t[m, o]    (ACT mul + DVE add)
where WFULL folds the final Linear:  b_t = conv @ W[:,1:].T,
a_t = [dens * W[:,0] + b | dens].  Psi_aug columns are host-permuted so that
chunk kk of 128 contiguous columns holds original targets m = 8p + kk,
giving a contiguous 2 KB-per-partition output DMA.

Sharded data-parallel over B across 8 NeuronCores (2 batches per core).
Falls back to a general (exact RBF) path if the low-rank preconditions do
not hold (per-channel length scales, or inputs outside the Chebyshev domain).
"""

import sys

if "/opt/trn_rl_repo" not in sys.path:
    sys.path.insert(0, "/opt/trn_rl_repo")

import numpy as np
import ml_dtypes

import concourse.bass as bass
import concourse.bacc as bacc
import concourse.tile as tile
import concourse.mybir as mybir
from concourse.bass_utils import run_bass_kernel_spmd

B, N, M, CIN, COUT = 16, 512, 1024, 7, 64
C = CIN + 1
N_CORES = 8
BPC = B // N_CORES  # batches per core
NT = N // 128       # n-tiles per batch
F32 = mybir.dt.float32
F32R = mybir.dt.float32r
F16 = mybir.dt.float16
BF16 = mybir.dt.bfloat16

# ---------------- low-rank fast path parameters ----------------
LR_RANK = 16          # rank R of the factored kernel
LR_DEG = 48           # Chebyshev degree of the interpolant
LR_DOM = (-0.01, 1.01)  # interpolation domain (must contain all c, t)
RA = LR_RANK + 1      # Psi rows incl. the constant (ones) row
NCH = 9               # ctx channels: ones, 7 features, zero pad


def _build_lr(reps=1, band=False, act_mul=True, act_copy=True):
    """Low-rank SPMD program v2: DMA-count-minimized.

    Per iteration (= BPC batches): one merged phi+ctx input DMA (DVE ring),
    one psi DMA per batch (SP ring), one merged output DMA (ACT ring).
    Final matmuls stream the full [RA, 129] PWA block per 128-target chunk;
    division runs in waves of <=3 chunks (PSUM bank limit).
    """
    del band, act_mul, act_copy  # v2 has a single layout
    nc = bacc.Bacc("TRN2", target_bir_lowering=False, debug=False,
                   num_devices=N_CORES)
    R = LR_RANK
    PC = R + NCH          # phi + ctx columns per n-tile
    AB = 2 * COUT + 1     # merged b|a|dens columns
    WAVES = [(0, 3), (3, 3), (6, 2)]  # (first chunk, n chunks)

    pc_d = nc.dram_tensor("pc", [128, BPC * NT * PC], F16,
                          kind="ExternalInput")
    psi_d = nc.dram_tensor("psi", [BPC, RA, M], F16, kind="ExternalInput")
    wf_d = nc.dram_tensor("wf", [NCH, AB], F16, kind="ExternalInput")
    parow_d = nc.dram_tensor("parow", [1, AB], F16, kind="ExternalInput")
    out_d = nc.dram_tensor("out", [BPC, M, COUT], F32, kind="ExternalOutput")

    with tile.TileContext(nc) as tc:
        with (
            tc.tile_pool(name="const", bufs=1) as constp,
            tc.tile_pool(name="inp", bufs=2) as inp,
            tc.tile_pool(name="psip", bufs=3) as psip,
            tc.tile_pool(name="small", bufs=2) as smallp,
            tc.tile_pool(name="resp", bufs=2) as resp,
            tc.tile_pool(name="rcp", bufs=3) as rcp,
            tc.tile_pool(name="pttps", bufs=2, space=bass.MemorySpace.PSUM) as pttps,
            tc.tile_pool(name="pwaps", bufs=2, space=bass.MemorySpace.PSUM) as pwaps,
            tc.tile_pool(name="abps", bufs=3, space=bass.MemorySpace.PSUM) as abps,
        ):
            wf_t = constp.tile([NCH, AB], F16, tag="wf")
            nc.sync.dma_start(wf_t[:], wf_d.ap())

            def emit_iter(it):
                pc_t = inp.tile([128, BPC, NT, PC], F16, tag="pc")
                nc.gpsimd.dma_start(
                    pc_t[:], pc_d.ap().rearrange("p (b k c) -> p b k c",
                                                 b=BPC, k=NT))
                psi_ts, ab_all = [], []
                for j in range(BPC):
                    psi_t = psip.tile([RA, M], F16, tag=f"psi{j}")
                    nc.sync.dma_start(psi_t[:], psi_d.ap()[j])
                    psi_ts.append(psi_t)

                for j in range(BPC):
                    # P_Tt[c', r] = sum_n ctx[n, c'] * phi[n, r]
                    ptt_ps = pttps.tile([NCH, R], F32, tag="ptt")
                    for k in range(NT):
                        nc.tensor.matmul(ptt_ps[:], pc_t[:, j, k, R:R + NCH],
                                         pc_t[:, j, k, 0:R],
                                         start=(k == 0), stop=(k == NT - 1))
                    # hi/lo f16 split of P for a near-exact f16 PWA matmul
                    ptt_hi = smallp.tile([NCH, R], F16, tag="ptthi")
                    nc.scalar.copy(ptt_hi[:], ptt_ps[:])
                    ptt_lo = smallp.tile([NCH, R], F16, tag="pttlo")
                    nc.vector.tensor_tensor(ptt_lo[:], ptt_ps[:], ptt_hi[:],
                                            mybir.AluOpType.subtract)

                    pwa_ps = pwaps.tile([R, AB], F32, tag="pwa")
                    nc.tensor.matmul(pwa_ps[:], ptt_hi[:], wf_t[:],
                                     start=True, stop=False)
                    nc.tensor.matmul(pwa_ps[:], ptt_lo[:], wf_t[:],
                                     start=False, stop=True)
                    pwa_sb = smallp.tile([RA, AB], F16, tag="pwasb")
                    nc.scalar.copy(pwa_sb[0:R, :], pwa_ps[:])
                    if it < 1:
                        # constant row: written once per pool buffer and
                        # reused by all later iterations
                        nc.sync.dma_start(pwa_sb[R:RA, :], parow_d.ap())
                    ab_all.append(
                        [self_ab(j, w0, nw, psi_ts[j], pwa_sb)
                         for (w0, nw) in WAVES])

                res_t = resp.tile([128, BPC * 8 * COUT], F32, tag="res")
                for j in range(BPC):
                    for wi, (w0, nw) in enumerate(WAVES):
                        ab_ps = ab_all[j][wi]
                        ab_g = (ab_ps[:, 0:nw * AB]
                                .rearrange("p (g x) -> p g x", x=AB))
                        recip_t = rcp.tile([128, 3], F32, tag="recip")
                        nc.vector.reciprocal(recip_t[:, 0:nw],
                                             ab_g[:, :, 2 * COUT])
                        for g in range(nw):
                            kk = w0 + g
                            nc.scalar.mul(
                                res_t[:, (j * 8 + kk) * COUT:
                                      (j * 8 + kk + 1) * COUT],
                                ab_ps[:, g * AB:g * AB + COUT],
                                recip_t[:, g:g + 1])
                        res_g = (res_t[:, (j * 8 + w0) * COUT:
                                       (j * 8 + w0 + nw) * COUT]
                                 .rearrange("p (g x) -> p g x", x=COUT))
                        nc.vector.tensor_add(res_g, res_g,
                                             ab_g[:, :, COUT:2 * COUT])
                nc.scalar.dma_start(
                    out_d.ap().rearrange("b (p q) o -> p b q o", q=8),
                    res_t[:].rearrange("p (b q o) -> p b q o",
                                       b=BPC, q=8))

            def self_ab(j, w0, nw, psi_t, pwa_sb):
                ab_ps = abps.tile([128, 3 * AB], F32, tag="ab")
                for g in range(nw):
                    kk = w0 + g
                    nc.tensor.matmul(
                        ab_ps[:, g * AB:(g + 1) * AB],
                        psi_t[:, kk * 128:(kk + 1) * 128],
                        pwa_sb[:], start=True, stop=True)
                return ab_ps

            for it in range(reps):
                emit_iter(it)

    nc.compile()
    return nc


def _cheb_feats(x, deg, a, b):
    u = (2.0 * x - (a + b)) / (b - a)
    T = np.zeros(x.shape + (deg,))
    T[..., 0] = 1.0
    if deg > 1:
        T[..., 1] = u
    for i in range(2, deg):
        T[..., i] = 2.0 * u * T[..., i - 1] - T[..., i - 2]
    return T


def _lr_factors(s):
    """SVD-truncated Chebyshev factorization of exp(s*(c-t)^2) on LR_DOM."""
    a, b = LR_DOM
    D, R = LR_DEG, LR_RANK
    k = np.arange(D)
    z = (a + b) / 2 + (b - a) / 2 * np.cos((2 * k + 1) * np.pi / (2 * D))
    V = _cheb_feats(z, D, a, b)
    Vinv = np.linalg.inv(V)
    E = np.exp(s * (z[:, None] - z[None, :]) ** 2)
    K = Vinv @ E @ Vinv.T
    U, S, Vt = np.linalg.svd(K)
    FL = U[:, :R] * np.sqrt(S[:R])
    FR = Vt[:R].T * np.sqrt(S[:R])
    return FL, FR


def _host_prep_lr(context_in, context_out, target_in, sigma, W, b):
    ci = np.asarray(context_in, np.float64)[:, :, 0]
    ti = np.asarray(target_in, np.float64)[:, :, 0]
    co = np.asarray(context_out, np.float32)
    W = np.asarray(W, np.float64)
    bb = np.asarray(b, np.float64)
    s = float(-0.5 / np.exp(np.float64(np.asarray(sigma)[0])) ** 2)
    FL, FR = _lr_factors(s)
    R = LR_RANK
    PC = R + NCH
    AB = 2 * COUT + 1

    a, bdom = LR_DOM
    phit = (_cheb_feats(ci, LR_DEG, a, bdom) @ FL).astype(np.float16)
    psit = (_cheb_feats(ti, LR_DEG, a, bdom) @ FR).astype(np.float16)

    # merged phi|ctx layout: [B, 128, NT, PC] with cols 0:R = phi,
    # R:R+NCH = (ones, co, zero-pad)
    pc = np.zeros((B, N, PC), np.float16)
    pc[:, :, 0:R] = phit
    pc[:, :, R] = 1.0
    pc[:, :, R + 1:R + 8] = co
    pc = np.ascontiguousarray(
        pc.reshape(B, NT, 128, PC).transpose(0, 2, 1, 3))

    perm = np.array([8 * p + kk for kk in range(8) for p in range(128)])
    psi = np.zeros((B, RA, M), np.float16)
    psi[:, :R, :] = np.transpose(psit, (0, 2, 1))[:, :, perm]
    psi[:, R, :] = 1.0

    wf = np.zeros((NCH, AB), np.float16)
    wf[1:8, 0:COUT] = W[:, 1:C].T
    wf[0, COUT:2 * COUT] = W[:, 0]
    wf[0, 2 * COUT] = 1.0
    parow = np.zeros((1, AB), np.float16)
    parow[0, COUT:2 * COUT] = bb

    in_maps = []
    for core in range(N_CORES):
        sl = slice(core * BPC, (core + 1) * BPC)
        in_maps.append({
            "pc": np.ascontiguousarray(
                pc[sl].transpose(1, 0, 2, 3).reshape(128, BPC * NT * PC)),
            "psi": np.ascontiguousarray(psi[sl]),
            "wf": wf,
            "parow": parow,
        })
    return in_maps


def _lr_eligible(context_in, target_in, sigma):
    sig = np.asarray(sigma, np.float64)
    scales = np.exp(sig)
    if not np.all(scales == scales[0]):
        return False
    a, bdom = LR_DOM
    ci = np.asarray(context_in, np.float64)
    ti = np.asarray(target_in, np.float64)
    lo, hi = min(ci.min(), ti.min()), max(ci.max(), ti.max())
    if lo < a or hi > bdom:
        return False
    # the factorization needs the kernel to be resolvable at degree LR_DEG
    if -0.5 / scales[0] ** 2 < -400.0:
        return False
    return True


# =====================================================================
# Fallback path: exact RBF kernel (original implementation)
# =====================================================================

# (lhsT-part, rhs-part) index pairs for the bf16 3-way-split cross terms
_SPLIT_PAIRS = [(0, 0), (0, 1), (1, 0), (0, 2), (2, 0), (1, 1), (1, 2), (2, 1)]
SPLIT_K = 6 + len(_SPLIT_PAIRS)


def _build(svals, diff_mode, mm1_mode, epi_bcast, reps=1, diff_pack=True):
    """Build the SPMD Bass program. svals: tuple of per-group exp scales."""
    G = len(svals)
    KD = SPLIT_K if diff_mode == "split" else 3
    d_dt = BF16 if diff_mode == "split" else F32
    w_dt = {"f16": F16, "f32r": F32R, "f32": F32}[mm1_mode]

    nc = bacc.Bacc("TRN2", target_bir_lowering=False, debug=False,
                   num_devices=N_CORES)

    ctx_io_dt = F32 if mm1_mode == "f32r" else w_dt
    if diff_pack:
        L_d = nc.dram_tensor("L", [BPC, 32 + KD, NT // 2, 128], d_dt,
                             kind="ExternalInput")
        R_d = nc.dram_tensor("R", [BPC, 32 + KD, M], d_dt,
                             kind="ExternalInput")
    else:
        L_d = nc.dram_tensor("L", [BPC, KD, NT, 128], d_dt,
                             kind="ExternalInput")
        R_d = nc.dram_tensor("R", [BPC, KD, M], d_dt, kind="ExternalInput")
    ctx_d = nc.dram_tensor("ctx", [BPC, 128, G * NT * C], ctx_io_dt,
                           kind="ExternalInput")
    ones_d = nc.dram_tensor("ones", [1, M], F32, kind="ExternalInput")
    rb_d = nc.dram_tensor("rb", [C + 1, COUT], F32, kind="ExternalInput")
    ra_d = nc.dram_tensor("ra", [C + 1, COUT + 1], F32, kind="ExternalInput")
    out_d = nc.dram_tensor("out", [BPC, M, COUT], F32, kind="ExternalOutput")

    def mm_cast_mm1(ap):
        return ap.bitcast(F32R) if mm1_mode == "f32r" else ap

    with tile.TileContext(nc) as tc:
        with (
            tc.tile_pool(name="const", bufs=1) as constp,
            tc.tile_pool(name="inp", bufs=2) as inp,
            tc.tile_pool(name="wp", bufs=3) as wp,
            tc.tile_pool(name="o1p", bufs=2) as o1p,
            tc.tile_pool(name="resp", bufs=2) as resp,
            tc.tile_pool(name="rcp", bufs=2) as rcp,
            tc.tile_pool(name="dps", bufs=2, space=bass.MemorySpace.PSUM) as dps,
            tc.tile_pool(name="o1ps", bufs=1, space=bass.MemorySpace.PSUM) as o1ps,
            tc.tile_pool(name="aps", bufs=1, space=bass.MemorySpace.PSUM) as aps,
            tc.tile_pool(name="bps", bufs=1, space=bass.MemorySpace.PSUM) as bps,
        ):
            rb_t = constp.tile([C + 1, COUT], F32, tag="rb")
            nc.sync.dma_start(rb_t[:], rb_d.ap())
            ra_t = constp.tile([C + 1, COUT + 1], F32, tag="ra")
            nc.sync.dma_start(ra_t[:], ra_d.ap())

            def emit_phase1(j):
                lshape = ([32 + KD, NT // 2, 128] if diff_pack
                          else [KD, NT, 128])
                L_t = inp.tile(lshape, d_dt, tag="L")
                nc.sync.dma_start(L_t[:], L_d.ap()[j])
                R_t = inp.tile([32 + KD, M] if diff_pack else [KD, M],
                               d_dt, tag="R")
                nc.sync.dma_start(R_t[:], R_d.ap()[j])
                ctx_t = inp.tile([128, G, NT, C], ctx_io_dt, tag="ctx")
                nc.sync.dma_start(
                    ctx_t[:],
                    ctx_d.ap()[j].rearrange("p (g k c) -> p g k c",
                                            g=G, k=NT),
                )

                o1_t = o1ps.tile([C, M], F32, tag="o1")
                for k in range(NT):
                    base = 32 * (k % 2) if diff_pack else 0
                    d_t = dps.tile([128, M], F32, tag="d")
                    lhsT = (L_t[base:base + KD, k // 2, :] if diff_pack
                            else L_t[:, k, :])
                    for h in range(2):
                        nc.tensor.matmul(
                            d_t[:, h * 512:(h + 1) * 512],
                            lhsT,
                            R_t[base:base + KD, h * 512:(h + 1) * 512],
                            start=True, stop=True,
                            tile_position=(base, 0) if diff_pack else None,
                        )
                    for g in range(G):
                        w_t = wp.tile([128, M], w_dt, tag="w")
                        nc.scalar.activation(
                            w_t[:], d_t[:],
                            mybir.ActivationFunctionType.Exp,
                            scale=float(svals[g]),
                        )
                        first = (k == 0 and g == 0)
                        last = (k == NT - 1 and g == G - 1)
                        for h in range(2):
                            nc.tensor.matmul(
                                o1_t[:, h * 512:(h + 1) * 512],
                                mm_cast_mm1(ctx_t[:, g, k, :]),
                                mm_cast_mm1(w_t[:, h * 512:(h + 1) * 512]),
                                start=first, stop=last,
                            )
                return j, o1_t

            def emit_epilogue(j, o1_t):
                o1_sb = o1p.tile([C + 1, M], F32, tag="o1sb")
                nc.vector.tensor_copy(o1_sb[0:C, :], o1_t[:])
                nc.sync.dma_start(o1_sb[C:C + 1, :], ones_d.ap())
                o1_g = o1_sb[:].rearrange("p (m q) -> p q m", q=8)

                res_t = resp.tile([128, 8 * COUT], F32, tag="res")
                for wave in range(2):
                    a_t = aps.tile([128, 4 * (COUT + 1)], F32, tag="a")
                    b_t = bps.tile([128, 4 * COUT], F32, tag="b")
                    for g4 in range(4):
                        kk = wave * 4 + g4
                        lhsT9 = o1_g[:, kk, :]
                        nc.tensor.matmul(
                            b_t[:, g4 * COUT:(g4 + 1) * COUT],
                            lhsT9, rb_t[:], start=True, stop=True,
                        )
                        nc.tensor.matmul(
                            a_t[:, g4 * (COUT + 1):(g4 + 1) * (COUT + 1)],
                            lhsT9, ra_t[:], start=True, stop=True,
                        )
                    a_g = a_t[:].rearrange("p (g x) -> p g x", x=COUT + 1)
                    recip_t = rcp.tile([128, 4], F32, tag="recip")
                    nc.vector.reciprocal(recip_t[:], a_g[:, :, COUT])
                    res_g = (res_t[:, wave * 4 * COUT:(wave + 1) * 4 * COUT]
                             .rearrange("p (g x) -> p g x", x=COUT))
                    if epi_bcast:
                        rb_ap = recip_t[:].unsqueeze(2).broadcast_to([128, 4, COUT])
                        nc.vector.tensor_tensor(
                            res_g, b_t[:].rearrange("p (g x) -> p g x", x=COUT),
                            rb_ap, mybir.AluOpType.mult,
                        )
                    else:
                        for g4 in range(4):
                            nc.vector.tensor_scalar_mul(
                                res_t[:, g4 * COUT:(g4 + 1) * COUT],
                                b_t[:, g4 * COUT:(g4 + 1) * COUT],
                                recip_t[:, g4:g4 + 1],
                            )
                    nc.vector.tensor_add(res_g, res_g, a_g[:, :, 0:COUT])
                nc.sync.dma_start(
                    out_d.ap()[j].rearrange("(p q) o -> p (q o)", q=8),
                    res_t[:],
                )

            pending = None
            for rep_j in range(reps * BPC):
                st = emit_phase1(rep_j % BPC)
                if pending is not None:
                    emit_epilogue(*pending)
                pending = st
            emit_epilogue(*pending)

    nc.compile()
    return nc


_CACHE = {}


def get_program(key, reps=1):
    full_key = key + (reps,)
    if full_key not in _CACHE:
        if key[0] == "lr":
            _CACHE[full_key] = _build_lr(reps, *key[1:])
        else:
            (_, svals, diff_mode, mm1_mode, epi_bcast, diff_pack) = key
            _CACHE[full_key] = _build(svals, diff_mode, mm1_mode, epi_bcast,
                                      reps, diff_pack)
    return _CACHE[full_key]


def _split3(x64):
    """Split float64 array into 3 bf16 arrays summing to ~fp32 accuracy."""
    parts = []
    r = x64.copy()
    for _ in range(3):
        p = r.astype(np.float32).astype(ml_dtypes.bfloat16)
        parts.append(p)
        r = r - p.astype(np.float64)
    return parts


def _host_prep(context_in, context_out, target_in, sigma, W, b,
               diff_mode, mm1_mode, diff_pack=True):
    ci = np.ascontiguousarray(np.asarray(context_in, np.float32)[:, :, 0])
    ti = np.ascontiguousarray(np.asarray(target_in, np.float32)[:, :, 0])
    co = np.asarray(context_out, np.float32)
    sig = np.asarray(sigma, np.float32)
    W = np.asarray(W, np.float32)
    bb = np.asarray(b, np.float32)

    scales = np.exp(sig.astype(np.float64))
    svals = (-0.5 / scales ** 2).astype(np.float32)
    uniq, inv = np.unique(svals, return_inverse=True)
    G = len(uniq)

    c64 = ci.astype(np.float64)
    t64 = ti.astype(np.float64)
    if diff_mode == "split":
        dt_np = ml_dtypes.bfloat16
        c_p = _split3(c64)
        t_p = _split3(t64)
        c2_p = _split3(c64 ** 2)
        t2_p = _split3(t64 ** 2)
        onesN = np.ones((B, N), dt_np)
        onesM = np.ones((B, M), dt_np)
        Lrows = c2_p + [onesN] * 3
        Rrows = [onesM] * 3 + t2_p
        for (i, jj) in _SPLIT_PAIRS:
            Lrows.append(c_p[i])
            Rrows.append((-2.0 * t_p[jj].astype(np.float32)).astype(dt_np))
        Lflat = np.stack(Lrows, axis=1)
        Rflat = np.stack(Rrows, axis=1)
        KD = SPLIT_K
    else:
        dt_np = np.float32
        Lflat = np.stack([c64 ** 2, -2.0 * c64, np.ones_like(c64)],
                         axis=1).astype(np.float32)
        Rflat = np.stack([np.ones_like(t64), t64, t64 ** 2],
                         axis=1).astype(np.float32)
        KD = 3
    Lt = Lflat.reshape(B, KD, NT, 128)
    if diff_pack:
        L = np.zeros((B, 32 + KD, NT // 2, 128), dt_np)
        R = np.zeros((B, 32 + KD, M), dt_np)
        for k in range(NT):
            base = 32 * (k % 2)
            L[:, base:base + KD, k // 2, :] = Lt[:, :, k, :]
        R[:, 0:KD, :] = Rflat
        R[:, 32:32 + KD, :] = Rflat
    else:
        L = np.ascontiguousarray(Lt)
        R = np.ascontiguousarray(Rflat)

    w_np = np.float16 if mm1_mode == "f16" else np.float32
    ctx = np.zeros((B, G, N, C), w_np)
    for ch in range(C):
        g = int(inv[ch])
        if ch == 0:
            ctx[:, g, :, C - 1] = 1.0
        else:
            ctx[:, g, :, ch - 1] = co[:, :, ch - 1].astype(w_np)
    ctx = np.ascontiguousarray(
        ctx.reshape(B, G, NT, 128, C).transpose(0, 3, 1, 2, 4)
        .reshape(B, 128, G * NT * C))

    rb = np.zeros((C + 1, COUT), np.float32)
    rb[0:CIN, :] = W[:, 1:C].T
    ra = np.zeros((C + 1, COUT + 1), np.float32)
    ra[C - 1, 0:COUT] = W[:, 0]
    ra[C - 1, COUT] = 1.0
    ra[C, 0:COUT] = bb
    ra[C, COUT] = 1e-8

    onesrow = np.ones((1, M), np.float32)

    in_maps = []
    for core in range(N_CORES):
        sl = slice(core * BPC, (core + 1) * BPC)
        in_maps.append({
            "L": np.ascontiguousarray(L[sl]),
            "R": np.ascontiguousarray(R[sl]),
            "ctx": np.ascontiguousarray(ctx[sl]),
            "ones": onesrow,
            "rb": rb,
            "ra": ra,
        })
    return uniq, in_maps


LR_BAND = False
LR_TILE_POS = True
LR_ACT_MUL = True
LR_ACT_COPY = True
DIFF_MODE = "split"
MM1_MODE = "f16"
EPI_BCAST = True
DIFF_PACK = False
USE_LR = True


def prepare(inputs):
    """Host prep for the active path. Returns (program_key, in_maps)."""
    context_in = inputs["context_in"]
    context_out = inputs["context_out"]
    target_in = inputs["target_in"]
    sigma, W, b = inputs["sigma"], inputs["W"], inputs["b"]
    if USE_LR and _lr_eligible(context_in, target_in, sigma):
        in_maps = _host_prep_lr(context_in, context_out, target_in,
                                sigma, W, b)
        return ("lr", LR_BAND, LR_ACT_MUL, LR_ACT_COPY), in_maps
    uniq, in_maps = _host_prep(context_in, context_out, target_in,
                               sigma, W, b, DIFF_MODE, MM1_MODE, DIFF_PACK)
    key = ("gen", tuple(np.float32(uniq).tolist()), DIFF_MODE, MM1_MODE,
           EPI_BCAST, DIFF_PACK)
    return key, in_maps


def kernel(context_in, context_out, target_in, sigma, W, b, trace=False):
    key, in_maps = prepare({
        "context_in": context_in, "context_out": context_out,
        "target_in": target_in, "sigma": sigma, "W": W, "b": b})
    nc = get_program(key, 1)
    res = run_bass_kernel_spmd(nc, in_maps, core_ids=list(range(N_CORES)),
                               trace=trace)
    out = np.concatenate([res.results[i]["out"] for i in range(N_CORES)],
                         axis=0)
    if trace:
        kernel.last_exec_time_ns = res.exec_time_ns
        kernel.last_results = res
    return out
